# revision 1
# baseline (speedup 1.0000x reference)
"""Trainium2 Bass kernel for nn_CombinedLoss: weighted BCE (9x9 morphology
boundary weights) + soft dice, data-parallel over 8 NeuronCores.

Self-contained: hardcodes shapes [32,1,1024,1024] f32 and the sharding
(4 samples per core). Host combines tiny per-core partial sums.
"""

import numpy as np
import ml_dtypes

import concourse.bass as bass
import concourse.tile as tile
from concourse import bacc, mybir
from concourse.bass_utils import run_bass_kernel_spmd

AF = mybir.ActivationFunctionType
OP = mybir.AluOpType
BF16 = mybir.dt.bfloat16
F32 = mybir.dt.float32

B, H, W = 32, 1024, 1024
N_CORES = 8
SPC = B // N_CORES          # samples per core = 4
NB = H // 128               # 8 row blocks per sample
PAD = 8
SEG = W + 2 * PAD           # 1040 padded segment stride
MEGA = NB * SEG             # 8320
MEGAX = MEGA + 8            # +8 tail so shifted reads stay in-bounds
NSLOT = SPC * NB            # 32 accum slots per core
BOUNDARY_WEIGHT = 3.0
SMOOTH = 1.0

NPBF16 = ml_dtypes.bfloat16


def _cnt_v(r):
    return min(r, 4) + 1 + min(H - 1 - r, 4)


def _make_consts():
    k = np.arange(128)
    band9 = (np.abs(k[:, None] - k[None, :]) <= 4).astype(NPBF16)
    ht = np.zeros((128, 128), NPBF16)   # rows from PREV block (top halo)
    hb = np.zeros((128, 128), NPBF16)   # rows from NEXT block (bottom halo)
    for m in range(4):
        ht[124 + m:, m] = 1.0
    for m in range(124, 128):
        hb[: m - 123, m] = 1.0
    ones = np.ones((128, 1), NPBF16)
    # horizontal clipped-window rescale: data cols 0..3 and 1020..1023
    el = np.tile(np.array([9 / 5, 9 / 6, 9 / 7, 9 / 8], np.float32), NB)
    er = np.tile(np.array([9 / 8, 9 / 7, 9 / 6, 9 / 5], np.float32), NB)
    edgeL = np.broadcast_to(el, (128, NB * 4)).astype(NPBF16).copy()
    edgeR = np.broadcast_to(er, (128, NB * 4)).astype(NPBF16).copy()
    tau = np.zeros((128, NB), np.float32)
    for b in range(NB):
        for p in range(128):
            tau[p, b] = 9.0 * _cnt_v(b * 128 + p) - 0.5
    return {
        "band9": band9, "halo_t": ht, "halo_b": hb, "ones": ones,
        "edgeL": edgeL, "edgeR": edgeR, "tau": tau,
    }


def _build_module():
    nc = bacc.Bacc("TRN2", target_bir_lowering=False, debug=False,
                   num_devices=N_CORES)

    lg = nc.dram_tensor("lg", [SPC, H, W], F32, kind="ExternalInput").ap()
    tg = nc.dram_tensor("tg", [SPC, H, W], F32, kind="ExternalInput").ap()
    band9 = nc.dram_tensor("band9", [128, 128], BF16, kind="ExternalInput").ap()
    halo_t = nc.dram_tensor("halo_t", [128, 128], BF16, kind="ExternalInput").ap()
    halo_b = nc.dram_tensor("halo_b", [128, 128], BF16, kind="ExternalInput").ap()
    onesd = nc.dram_tensor("ones", [128, 1], BF16, kind="ExternalInput").ap()
    edgeL = nc.dram_tensor("edgeL", [128, NB * 4], BF16, kind="ExternalInput").ap()
    edgeR = nc.dram_tensor("edgeR", [128, NB * 4], BF16, kind="ExternalInput").ap()
    taud = nc.dram_tensor("tau", [128, NB], F32, kind="ExternalInput").ap()

    o_bce = nc.dram_tensor("o_bce", [128, NSLOT], F32, kind="ExternalOutput").ap()
    o_ga = nc.dram_tensor("o_ga", [128, NSLOT], F32, kind="ExternalOutput").ap()
    o_gb = nc.dram_tensor("o_gb", [128, NSLOT], F32, kind="ExternalOutput").ap()
    o_sig = nc.dram_tensor("o_sig", [128, NSLOT], F32, kind="ExternalOutput").ap()
    o_sigt = nc.dram_tensor("o_sigt", [128, NSLOT], F32, kind="ExternalOutput").ap()
    o_tsum = nc.dram_tensor("o_tsum", [1, SPC * 512], F32, kind="ExternalOutput").ap()

    with tile.TileContext(nc) as tc:
        with (
            tc.tile_pool(name="const", bufs=1) as cpool,
            tc.tile_pool(name="io", bufs=2) as iopool,
            tc.tile_pool(name="mega", bufs=1) as mpool,
            tc.tile_pool(name="blk", bufs=3) as bpool,
            tc.tile_pool(name="acc", bufs=1) as apool,
            tc.tile_pool(name="ps", bufs=2, space="PSUM") as pspool,
            tc.tile_pool(name="pst", bufs=1, space="PSUM") as pstpool,
        ):
            # ---- constants into SBUF
            band9_t = cpool.tile([128, 128], BF16, tag="band9")
            halo_t_t = cpool.tile([128, 128], BF16, tag="halo_t")
            halo_b_t = cpool.tile([128, 128], BF16, tag="halo_b")
            ones_t = cpool.tile([128, 1], BF16, tag="ones")
            edgeL_t = cpool.tile([128, NB * 4], BF16, tag="edgeL")
            edgeR_t = cpool.tile([128, NB * 4], BF16, tag="edgeR")
            tau_t = cpool.tile([128, NB], F32, tag="tau")
            nc.sync.dma_start(out=band9_t[:], in_=band9)
            nc.sync.dma_start(out=halo_t_t[:], in_=halo_t)
            nc.sync.dma_start(out=halo_b_t[:], in_=halo_b)
            nc.sync.dma_start(out=ones_t[:], in_=onesd)
            nc.sync.dma_start(out=edgeL_t[:], in_=edgeL)
            nc.sync.dma_start(out=edgeR_t[:], in_=edgeR)
            nc.sync.dma_start(out=tau_t[:], in_=taud)

            # ---- accumulator staging (every slot written exactly once)
            a_bce = apool.tile([128, NSLOT], F32, tag="a_bce")
            a_ga = apool.tile([128, NSLOT], F32, tag="a_ga")
            a_gb = apool.tile([128, NSLOT], F32, tag="a_gb")
            a_sig = apool.tile([128, NSLOT], F32, tag="a_sig")
            a_sigt = apool.tile([128, NSLOT], F32, tag="a_sigt")
            t_stage = apool.tile([1, SPC * 512], F32, tag="t_stage")

            # ---- persistent chain + per-sample megatiles
            cA = mpool.tile([128, MEGAX], BF16, tag="cA")
            cB = mpool.tile([128, MEGAX], BF16, tag="cB")
            nc.vector.memset(cA[:, MEGA:MEGAX], 0.0)
            nc.vector.memset(cB[:, MEGA:MEGAX], 0.0)

            for s in range(SPC):
                Lt = iopool.tile([128, MEGAX], BF16, tag="Lt")
                Tt = iopool.tile([128, MEGAX], BF16, tag="Tt")
                Tt3 = Tt[:, 0:MEGA].rearrange("p (b c) -> p b c", c=SEG)
                Lt3 = Lt[:, 0:MEGA].rearrange("p (b c) -> p b c", c=SEG)
                # zero pads of targets (morphology reads them)
                nc.vector.memset(Tt3[:, :, 0:PAD], 0.0)
                nc.vector.memset(Tt3[:, :, PAD + W:SEG], 0.0)
                nc.vector.memset(Tt[:, MEGA:MEGAX], 0.0)
                # cast loads f32 -> bf16 (SWDGE)
                nc.gpsimd.dma_start(
                    out=Lt3[:, :, PAD:PAD + W],
                    in_=lg[s].rearrange("(b p) w -> p b w", p=128))
                nc.gpsimd.dma_start(
                    out=Tt3[:, :, PAD:PAD + W],
                    in_=tg[s].rearrange("(b p) w -> p b w", p=128))

                # ---- horizontal 9-box-sum (log chain), width MEGA
                CW = MEGA
                nc.vector.tensor_add(cA[:, 0:CW], Tt[:, 0:CW], Tt[:, 1:CW + 1])
                nc.vector.tensor_add(cB[:, 0:CW], cA[:, 0:CW], cA[:, 2:CW + 2])
                nc.vector.tensor_add(cA[:, 0:CW], cB[:, 0:CW], cB[:, 4:CW + 4])
                nc.vector.tensor_add(cB[:, 0:CW], cA[:, 0:CW], Tt[:, 8:CW + 8])
                # edge rescale (clipped horizontal windows)
                s9v = cB[:, 0:MEGA].rearrange("p (b c) -> p b c", c=SEG)
                eL3 = edgeL_t[:].rearrange("p (b c) -> p b c", c=4)
                eR3 = edgeR_t[:].rearrange("p (b c) -> p b c", c=4)
                nc.vector.tensor_mul(s9v[:, :, 4:8], s9v[:, :, 4:8], eL3)
                nc.vector.tensor_mul(s9v[:, :, 1024:1028],
                                     s9v[:, :, 1024:1028], eR3)

                # ---- z = x*(2t-1); bce = -ln(sigmoid(z))
                zt = mpool.tile([128, MEGA], BF16, tag="zt")
                for b in range(NB):
                    o = b * SEG + PAD
                    nc.vector.tensor_scalar(
                        out=zt[:, o:o + W], in0=Tt[:, o:o + W],
                        scalar1=2.0, scalar2=-1.0, op0=OP.mult, op1=OP.add)
                    nc.vector.tensor_mul(zt[:, o:o + W], zt[:, o:o + W],
                                         Lt[:, o:o + W])

                # ---- ACT batch 1 (sigmoid set): sigma(z), sigma(x)
                sgz = mpool.tile([128, MEGA], BF16, tag="sgz")
                for b in range(NB):
                    o = b * SEG + PAD
                    nc.scalar.activation(sgz[:, o:o + W], zt[:, o:o + W],
                                         AF.Sigmoid)
                for b in range(NB):
                    o = b * SEG + PAD
                    slot = s * NB + b
                    junk0 = bpool.tile([128, W], BF16, tag="junk")
                    nc.scalar.activation(junk0[:], Lt[:, o:o + W],
                                         AF.Sigmoid,
                                         accum_out=a_sig[:, slot:slot + 1])
                # ---- ACT batch 2 (ln set): mln = ln(sigmoid(z)) = -bce
                mln = mpool.tile([128, MEGA], BF16, tag="mln")
                for b in range(NB):
                    o = b * SEG + PAD
                    slot = s * NB + b
                    nc.scalar.activation(mln[:, o:o + W], sgz[:, o:o + W],
                                         AF.Ln,
                                         accum_out=a_bce[:, slot:slot + 1])

                bce = mln
                tsum = pstpool.tile([1, 512], F32, tag="tsum")

                for b in range(NB):
                    o = b * SEG + PAD
                    slot = s * NB + b
                    # sigma(x) * t == sigma(z) * t ; accum -> intersection
                    junk = bpool.tile([128, W], BF16, tag="junk")
                    nc.vector.scalar_tensor_tensor(
                        out=junk[:], in0=Tt[:, o:o + W], scalar=1.0,
                        in1=sgz[:, o:o + W], op0=OP.mult, op1=OP.mult,
                        accum_out=a_sigt[:, slot:slot + 1])

                    # ---- vertical 9-box-sum via banded matmuls into PSUM
                    S2D = pspool.tile([128, W], F32, tag="S2D")
                    for h_ in range(2):
                        rb = b * SEG + 4 + h_ * 512
                        outp = S2D[:, h_ * 512:(h_ + 1) * 512]
                        mm = [(band9_t, b)]
                        if b > 0:
                            mm.append((halo_t_t, b - 1))
                        if b < NB - 1:
                            mm.append((halo_b_t, b + 1))
                        for i, (wt, bb) in enumerate(mm):
                            rc = bb * SEG + 4 + h_ * 512
                            nc.tensor.matmul(
                                outp, wt[:], cB[:, rc:rc + 512],
                                start=(i == 0), stop=(i == len(mm) - 1))

                    # ---- masked bce sums from S2D thresholds
                    junk2 = bpool.tile([128, W], BF16, tag="junk")
                    nc.vector.scalar_tensor_tensor(
                        out=junk2[:], in0=S2D[:], scalar=tau_t[:, b:b + 1],
                        in1=bce[:, o:o + W], op0=OP.is_lt, op1=OP.mult,
                        accum_out=a_ga[:, slot:slot + 1])
                    junk3 = bpool.tile([128, W], BF16, tag="junk")
                    nc.vector.scalar_tensor_tensor(
                        out=junk3[:], in0=S2D[:], scalar=0.5,
                        in1=bce[:, o:o + W], op0=OP.is_lt, op1=OP.mult,
                        accum_out=a_gb[:, slot:slot + 1])

                    # ---- sum(t) via ones-matmul, accumulated over the sample
                    nc.tensor.matmul(tsum[:], ones_t[:], Tt[:, o:o + 512],
                                     start=(b == 0), stop=False)
                    nc.tensor.matmul(tsum[:], ones_t[:],
                                     Tt[:, o + 512:o + 1024],
                                     start=False, stop=(b == NB - 1))

                nc.vector.tensor_copy(
                    out=t_stage[0:1, s * 512:(s + 1) * 512], in_=tsum[0:1, :])

            nc.sync.dma_start(out=o_bce, in_=a_bce[:])
            nc.sync.dma_start(out=o_ga, in_=a_ga[:])
            nc.sync.dma_start(out=o_gb, in_=a_gb[:])
            nc.sync.dma_start(out=o_sig, in_=a_sig[:])
            nc.sync.dma_start(out=o_sigt, in_=a_sigt[:])
            nc.sync.dma_start(out=o_tsum, in_=t_stage[:])
    nc.finalize()
    return nc


_NC = None


def _get_module():
    global _NC
    if _NC is None:
        _NC = _build_module()
    return _NC


def _run(logits, targets, trace=False):
    lg = np.ascontiguousarray(np.asarray(logits, np.float32).reshape(B, H, W))
    tg = np.ascontiguousarray(np.asarray(targets, np.float32).reshape(B, H, W))
    consts = _make_consts()
    nc = _get_module()
    in_maps = []
    for c in range(N_CORES):
        m = dict(consts)
        m["lg"] = lg[c * SPC:(c + 1) * SPC]
        m["tg"] = tg[c * SPC:(c + 1) * SPC]
        in_maps.append(m)
    res = run_bass_kernel_spmd(nc, in_maps, core_ids=list(range(N_CORES)),
                               trace=trace)
    return res


def _combine(results):
    wb = 0.0
    scores = []
    for c in range(N_CORES):
        r = results[c]
        # o_bce/o_ga/o_gb hold sums of ln(sigmoid(z)) = -bce (masked for ga/gb)
        bce_s = -r["o_bce"].astype(np.float64).sum()
        ga = -r["o_ga"].astype(np.float64).sum()
        gb = -r["o_gb"].astype(np.float64).sum()
        wb += bce_s + (BOUNDARY_WEIGHT - 1.0) * (ga - gb)
        for s in range(SPC):
            P = r["o_sig"][:, s * NB:(s + 1) * NB].astype(np.float64).sum()
            I = r["o_sigt"][:, s * NB:(s + 1) * NB].astype(np.float64).sum()
            T = r["o_tsum"][0, s * 512:(s + 1) * 512].astype(np.float64).sum()
            scores.append(2.0 * (I + SMOOTH) / (P + T + SMOOTH))
    bce = wb / (B * H * W)
    dice = 1.0 - np.mean(scores)
    return np.float32(bce + dice)


def kernel(logits, targets):
    res = _run(logits, targets, trace=False)
    return _combine(res.results)



# revision 2
# speedup vs baseline: 1.0888x; 1.0888x over previous
"""Trainium2 Bass kernel for nn_CombinedLoss: weighted BCE (9x9 morphology
boundary weights) + soft dice, data-parallel over 8 NeuronCores.

Self-contained: hardcodes shapes [32,1,1024,1024] f32 and the sharding
(4 samples per core). Host combines tiny per-core partial sums.

V2 design notes (per core, per 128-row block of 1024 cols):
 - horizontal 9-box-sum of targets: 4 bf16 tensor_tensor adds (DVE 2x)
 - vertical 9-box-sum: banded matmuls into PSUM (TensorE), LDWEIGHTS
   minimized by looping chunks inside each stationary
 - boundary mask combined into ONE test: boundary <=> 0.5 < S2D < tau
   <=> |S2D - c| < r with c=(tau+.5)/2=9*cnt_v/2, r=(tau-.5)/2.
   u=|S2D-c| computed by Scalar ACT Abs (bias=-c per partition), then
   one stt(is_lt,mult) accumulates sum(bce * mask).
 - z = x*(2t-1) via ts + tt; sigmoid & ln ACTs run over the whole
   megatile with accum (pads preset so they contribute exactly 1.0 to
   the sigmoid accum and ~0 to the ln accum).
 - dice needs only I = sum(t*sgz) and sum(sgz): P + T = 2I - sum(sgz)
   + HW (the target-sum cancels), so no ones-matmuls at all.
"""

import numpy as np
import ml_dtypes

import concourse.bass as bass
import concourse.tile as tile
from concourse import bacc, mybir
from concourse.bass_utils import run_bass_kernel_spmd

AF = mybir.ActivationFunctionType
OP = mybir.AluOpType
BF16 = mybir.dt.bfloat16
F32 = mybir.dt.float32

B, H, W = 32, 1024, 1024
N_CORES = 8
SPC = B // N_CORES          # samples per core = 4
NB = H // 128               # 8 row blocks per sample
PAD = 8
SEG = W + 2 * PAD           # 1040 padded segment stride
MEGA = NB * SEG             # 8320
MEGAX = MEGA + 8            # +8 tail so shifted reads stay in-bounds
NSLOT = SPC * NB            # 32 accum slots per core
BOUNDARY_WEIGHT = 3.0
SMOOTH = 1.0
ZPAD = 20.0                 # zt pad value: sigmoid(20) == 1.0 in bf16

NPBF16 = ml_dtypes.bfloat16


def _cnt_v(r):
    return min(r, 4) + 1 + min(H - 1 - r, 4)


def _make_consts():
    k = np.arange(128)
    band9 = (np.abs(k[:, None] - k[None, :]) <= 4).astype(NPBF16)
    ht = np.zeros((128, 128), NPBF16)   # rows from PREV block (top halo)
    hb = np.zeros((128, 128), NPBF16)   # rows from NEXT block (bottom halo)
    for m in range(4):
        ht[124 + m:, m] = 1.0
    for m in range(124, 128):
        hb[: m - 123, m] = 1.0
    # horizontal clipped-window rescale: data cols 0..3 and 1020..1023
    el = np.tile(np.array([9 / 5, 9 / 6, 9 / 7, 9 / 8], np.float32), NB)
    er = np.tile(np.array([9 / 8, 9 / 7, 9 / 6, 9 / 5], np.float32), NB)
    edgeL = np.broadcast_to(el, (128, NB * 4)).astype(NPBF16).copy()
    edgeR = np.broadcast_to(er, (128, NB * 4)).astype(NPBF16).copy()
    # combined-threshold constants: boundary <=> |S - c| < r
    cc = np.zeros((128, NB), np.float32)
    rr = np.zeros((128, NB), np.float32)
    for b in range(NB):
        for p in range(128):
            cv = 9.0 * _cnt_v(b * 128 + p)
            cc[p, b] = -cv / 2.0            # ACT bias = -c
            rr[p, b] = (cv - 1.0) / 2.0     # compare radius
    return {
        "band9": band9, "halo_t": ht, "halo_b": hb,
        "edgeL": edgeL, "edgeR": edgeR, "cbias": cc, "rrad": rr,
    }


def _build_module():
    nc = bacc.Bacc("TRN2", target_bir_lowering=False, debug=False,
                   num_devices=N_CORES)

    lg = nc.dram_tensor("lg", [SPC, H, W], F32, kind="ExternalInput").ap()
    tg = nc.dram_tensor("tg", [SPC, H, W], F32, kind="ExternalInput").ap()
    band9 = nc.dram_tensor("band9", [128, 128], BF16, kind="ExternalInput").ap()
    halo_t = nc.dram_tensor("halo_t", [128, 128], BF16, kind="ExternalInput").ap()
    halo_b = nc.dram_tensor("halo_b", [128, 128], BF16, kind="ExternalInput").ap()
    edgeL = nc.dram_tensor("edgeL", [128, NB * 4], BF16, kind="ExternalInput").ap()
    edgeR = nc.dram_tensor("edgeR", [128, NB * 4], BF16, kind="ExternalInput").ap()
    cbiasd = nc.dram_tensor("cbias", [128, NB], F32, kind="ExternalInput").ap()
    rradd = nc.dram_tensor("rrad", [128, NB], F32, kind="ExternalInput").ap()

    o_bce = nc.dram_tensor("o_bce", [128, SPC], F32, kind="ExternalOutput").ap()
    o_sgz = nc.dram_tensor("o_sgz", [128, SPC], F32, kind="ExternalOutput").ap()
    o_g = nc.dram_tensor("o_g", [128, NSLOT], F32, kind="ExternalOutput").ap()
    o_sigt = nc.dram_tensor("o_sigt", [128, NSLOT], F32, kind="ExternalOutput").ap()

    with tile.TileContext(nc) as tc:
        with (
            tc.tile_pool(name="const", bufs=1) as cpool,
            tc.tile_pool(name="io", bufs=2) as iopool,
            tc.tile_pool(name="mega", bufs=1) as mpool,
            tc.tile_pool(name="blk", bufs=3) as bpool,
            tc.tile_pool(name="acc", bufs=1) as apool,
            tc.tile_pool(name="ps", bufs=2, space="PSUM") as pspool,
        ):
            # ---- constants into SBUF
            band9_t = cpool.tile([128, 128], BF16, tag="band9")
            halo_t_t = cpool.tile([128, 128], BF16, tag="halo_t")
            halo_b_t = cpool.tile([128, 128], BF16, tag="halo_b")
            edgeL_t = cpool.tile([128, NB * 4], BF16, tag="edgeL")
            edgeR_t = cpool.tile([128, NB * 4], BF16, tag="edgeR")
            cbias_t = cpool.tile([128, NB], F32, tag="cbias")
            rrad_t = cpool.tile([128, NB], F32, tag="rrad")
            nc.sync.dma_start(out=band9_t[:], in_=band9)
            nc.sync.dma_start(out=halo_t_t[:], in_=halo_t)
            nc.sync.dma_start(out=halo_b_t[:], in_=halo_b)
            nc.sync.dma_start(out=edgeL_t[:], in_=edgeL)
            nc.sync.dma_start(out=edgeR_t[:], in_=edgeR)
            nc.sync.dma_start(out=cbias_t[:], in_=cbiasd)
            nc.sync.dma_start(out=rrad_t[:], in_=rradd)

            # ---- accumulator staging (every slot written exactly once)
            a_bce = apool.tile([128, SPC], F32, tag="a_bce")
            a_sgz = apool.tile([128, SPC], F32, tag="a_sgz")
            a_g = apool.tile([128, NSLOT], F32, tag="a_g")
            a_sigt = apool.tile([128, NSLOT], F32, tag="a_sigt")

            # ---- persistent chain buffers + per-sample megatiles
            cA = mpool.tile([128, MEGAX], BF16, tag="cA")
            cB = mpool.tile([128, MEGAX], BF16, tag="cB")
            nc.vector.memset(cA[:, MEGA:MEGAX], 0.0)
            nc.vector.memset(cB[:, MEGA:MEGAX], 0.0)
            zt = mpool.tile([128, MEGA], BF16, tag="zt")
            sgz = mpool.tile([128, MEGA], BF16, tag="sgz")
            mln = mpool.tile([128, MEGA], BF16, tag="mln")
            # zt pads preset once: sigmoid(+20) == 1.0 (bf16), ln(1.0) ~ 0
            zt3 = zt[:].rearrange("p (b c) -> p b c", c=SEG)
            nc.vector.memset(zt3[:, :, 0:PAD], ZPAD)
            nc.vector.memset(zt3[:, :, PAD + W:SEG], ZPAD)

            for s in range(SPC):
                Lt = iopool.tile([128, MEGAX], BF16, tag="Lt")
                Tt = iopool.tile([128, MEGAX], BF16, tag="Tt")
                Tt3 = Tt[:, 0:MEGA].rearrange("p (b c) -> p b c", c=SEG)
                Lt3 = Lt[:, 0:MEGA].rearrange("p (b c) -> p b c", c=SEG)
                # zero pads of targets (morphology reads them)
                nc.vector.memset(Tt3[:, :, 0:PAD], 0.0)
                nc.vector.memset(Tt3[:, :, PAD + W:SEG], 0.0)
                nc.vector.memset(Tt[:, MEGA:MEGAX], 0.0)
                # cast loads f32 -> bf16 (SWDGE)
                nc.gpsimd.dma_start(
                    out=Lt3[:, :, PAD:PAD + W],
                    in_=lg[s].rearrange("(b p) w -> p b w", p=128))
                nc.gpsimd.dma_start(
                    out=Tt3[:, :, PAD:PAD + W],
                    in_=tg[s].rearrange("(b p) w -> p b w", p=128))

                # ---- horizontal 9-box-sum (log chain), width MEGA
                CW = MEGA
                nc.vector.tensor_add(cA[:, 0:CW], Tt[:, 0:CW], Tt[:, 1:CW + 1])
                nc.vector.tensor_add(cB[:, 0:CW], cA[:, 0:CW], cA[:, 2:CW + 2])
                nc.vector.tensor_add(cA[:, 0:CW], cB[:, 0:CW], cB[:, 4:CW + 4])
                nc.vector.tensor_add(cB[:, 0:CW], cA[:, 0:CW], Tt[:, 8:CW + 8])
                # edge rescale (clipped horizontal windows)
                s9v = cB[:, 0:MEGA].rearrange("p (b c) -> p b c", c=SEG)
                eL3 = edgeL_t[:].rearrange("p (b c) -> p b c", c=4)
                eR3 = edgeR_t[:].rearrange("p (b c) -> p b c", c=4)
                nc.vector.tensor_mul(s9v[:, :, 4:8], s9v[:, :, 4:8], eL3)
                nc.vector.tensor_mul(s9v[:, :, 1024:1028],
                                     s9v[:, :, 1024:1028], eR3)

                # ---- z = x*(2t-1) per block (pads of zt stay +ZPAD)
                for b in range(NB):
                    o = b * SEG + PAD
                    nc.vector.tensor_scalar(
                        out=zt[:, o:o + W], in0=Tt[:, o:o + W],
                        scalar1=2.0, scalar2=-1.0, op0=OP.mult, op1=OP.add)
                    nc.vector.tensor_mul(zt[:, o:o + W], zt[:, o:o + W],
                                         Lt[:, o:o + W])

                # ---- Scalar ACTs over the full megatile with accum
                nc.scalar.activation(sgz[:], zt[:], AF.Sigmoid,
                                     accum_out=a_sgz[:, s:s + 1])
                nc.scalar.activation(mln[:], sgz[:], AF.Ln,
                                     accum_out=a_bce[:, s:s + 1])

                for b in range(NB):
                    o = b * SEG + PAD
                    slot = s * NB + b

                    # ---- vertical 9-box-sum via banded matmuls into PSUM
                    S2D = pspool.tile([128, W], F32, tag="S2D")
                    mm = [(band9_t, b)]
                    if b > 0:
                        mm.append((halo_t_t, b - 1))
                    if b < NB - 1:
                        mm.append((halo_b_t, b + 1))
                    for i, (wt, bb) in enumerate(mm):
                        for h_ in range(2):
                            rc = bb * SEG + 4 + h_ * 512
                            nc.tensor.matmul(
                                S2D[:, h_ * 512:(h_ + 1) * 512],
                                wt[:], cB[:, rc:rc + 512],
                                start=(i == 0), stop=(i == len(mm) - 1))

                    # ---- u = |S2D - c| on Scalar (PSUM -> SBUF bf16)
                    u = bpool.tile([128, W], BF16, tag="u")
                    nc.scalar.activation(u[:], S2D[:], AF.Abs,
                                         bias=cbias_t[:, b:b + 1], scale=1.0)

                    # ---- masked bce sum: sum(mln * [u < r])
                    junk = bpool.tile([128, W], BF16, tag="junk")
                    nc.vector.scalar_tensor_tensor(
                        out=junk[:], in0=u[:], scalar=rrad_t[:, b:b + 1],
                        in1=mln[:, o:o + W], op0=OP.is_lt, op1=OP.mult,
                        accum_out=a_g[:, slot:slot + 1])

                    # ---- intersection: sum(t * sgz)
                    junk2 = bpool.tile([128, W], BF16, tag="junk")
                    nc.vector.scalar_tensor_tensor(
                        out=junk2[:], in0=Tt[:, o:o + W], scalar=1.0,
                        in1=sgz[:, o:o + W], op0=OP.mult, op1=OP.mult,
                        accum_out=a_sigt[:, slot:slot + 1])

            nc.sync.dma_start(out=o_bce, in_=a_bce[:])
            nc.sync.dma_start(out=o_sgz, in_=a_sgz[:])
            nc.sync.dma_start(out=o_g, in_=a_g[:])
            nc.sync.dma_start(out=o_sigt, in_=a_sigt[:])
    nc.finalize()
    return nc


_NC = None


def _get_module():
    global _NC
    if _NC is None:
        _NC = _build_module()
    return _NC


def _run(logits, targets, trace=False):
    lg = np.ascontiguousarray(np.asarray(logits, np.float32).reshape(B, H, W))
    tg = np.ascontiguousarray(np.asarray(targets, np.float32).reshape(B, H, W))
    consts = _make_consts()
    nc = _get_module()
    in_maps = []
    for c in range(N_CORES):
        m = dict(consts)
        m["lg"] = lg[c * SPC:(c + 1) * SPC]
        m["tg"] = tg[c * SPC:(c + 1) * SPC]
        in_maps.append(m)
    res = run_bass_kernel_spmd(nc, in_maps, core_ids=list(range(N_CORES)),
                               trace=trace)
    return res


def _combine(results):
    HW = H * W
    wb = 0.0
    scores = []
    for c in range(N_CORES):
        r = results[c]
        # o_bce holds sum of ln(sigmoid(z)) = -bce (pads add ~ln(1)=0)
        # o_g holds sum of ln(sigmoid(z)) * boundary_mask
        bce_s = -r["o_bce"].astype(np.float64).sum()
        gm = -r["o_g"].astype(np.float64).sum()
        wb += bce_s + (BOUNDARY_WEIGHT - 1.0) * gm
        for s in range(SPC):
            I = r["o_sigt"][:, s * NB:(s + 1) * NB].astype(np.float64).sum()
            # sigmoid accum includes 128 pad cols of exactly 1.0 per row
            sgz_sum = r["o_sgz"][:, s].astype(np.float64).sum() - 128.0 * 128.0
            # P + T = 2I - sum(sgz) + HW  (target count cancels)
            scores.append(2.0 * (I + SMOOTH) / (2.0 * I - sgz_sum + HW + SMOOTH))
    bce = wb / (B * HW)
    dice = 1.0 - np.mean(scores)
    return np.float32(bce + dice)


def kernel(logits, targets):
    res = _run(logits, targets, trace=False)
    return _combine(res.results)


# revision 3
# speedup vs baseline: 1.2539x; 1.1517x over previous
"""Trainium2 Bass kernel for nn_CombinedLoss: weighted BCE (9x9 morphology
boundary weights) + soft dice, data-parallel over 8 NeuronCores.

Self-contained: hardcodes shapes [32,1,1024,1024] f32 and the sharding
(4 samples per core). Host combines tiny per-core partial sums.

V3 design notes (per core):
 - samples processed in pairs so the Scalar ACT table switches
   (sigmoid <-> ln sets) happen 2x per pair instead of 2x per sample.
 - z = x*(2t-1) built megatile-wide: ts(2t-1) -> pad re-memset(+20) ->
   tt(*x) with Lt pads preset 1.0, so pads land at +20 and contribute
   exactly 1.0 to the sigmoid accum (subtracted host-side) and ~0 to
   the ln accum.
 - boundary mask combined into ONE test: boundary <=> 0.5 < S2D < tau
   <=> |S2D - c| < r. u = |S2D - c| on Scalar (ACT Abs, bias per
   partition), then one stt(is_lt,mult) accumulates sum(bce * mask).
 - intersection I = sum(t*sgz): per-block product on DVE (2x tt), then
   ones-matmul column-sum accumulated in PSUM across the sample
   (TensorE), relieving the DVE.
 - dice denominator: P + T = 2I - sum(sgz) + HW (target count
   cancels), so no target-sum reduction is needed at all.
 - mln reuses the zt pool buffers (zt is dead once sigmoid ran).
"""

import numpy as np
import ml_dtypes

import concourse.bass as bass
import concourse.tile as tile
from concourse import bacc, mybir
from concourse.bass_utils import run_bass_kernel_spmd

AF = mybir.ActivationFunctionType
OP = mybir.AluOpType
BF16 = mybir.dt.bfloat16
F32 = mybir.dt.float32

B, H, W = 32, 1024, 1024
N_CORES = 8
SPC = B // N_CORES          # samples per core = 4
NB = H // 128               # 8 row blocks per sample
PAD = 8
SEG = W + 2 * PAD           # 1040 padded segment stride
MEGA = NB * SEG             # 8320
MEGAX = MEGA + 8            # +8 tail so shifted reads stay in-bounds
NSLOT = SPC * NB            # 32 accum slots per core
BOUNDARY_WEIGHT = 3.0
SMOOTH = 1.0
ZPAD = 20.0                 # zt pad value: sigmoid(20) == 1.0 in bf16

NPBF16 = ml_dtypes.bfloat16


def _cnt_v(r):
    return min(r, 4) + 1 + min(H - 1 - r, 4)


def _make_consts():
    k = np.arange(128)
    band9 = (np.abs(k[:, None] - k[None, :]) <= 4).astype(NPBF16)
    ht = np.zeros((128, 128), NPBF16)   # rows from PREV block (top halo)
    hb = np.zeros((128, 128), NPBF16)   # rows from NEXT block (bottom halo)
    for m in range(4):
        ht[124 + m:, m] = 1.0
    for m in range(124, 128):
        hb[: m - 123, m] = 1.0
    ones = np.ones((128, 1), NPBF16)
    # horizontal clipped-window rescale: data cols 0..3 and 1020..1023
    el = np.tile(np.array([9 / 5, 9 / 6, 9 / 7, 9 / 8], np.float32), NB)
    er = np.tile(np.array([9 / 8, 9 / 7, 9 / 6, 9 / 5], np.float32), NB)
    edgeL = np.broadcast_to(el, (128, NB * 4)).astype(NPBF16).copy()
    edgeR = np.broadcast_to(er, (128, NB * 4)).astype(NPBF16).copy()
    # combined-threshold constants: boundary <=> |S - c| < r
    cc = np.zeros((128, NB), np.float32)
    rr = np.zeros((128, NB), np.float32)
    for b in range(NB):
        for p in range(128):
            cv = 9.0 * _cnt_v(b * 128 + p)
            cc[p, b] = -cv / 2.0            # ACT bias = -c
            rr[p, b] = (cv - 1.0) / 2.0     # compare radius
    return {
        "band9": band9, "halo_t": ht, "halo_b": hb, "ones": ones,
        "edgeL": edgeL, "edgeR": edgeR, "cbias": cc, "rrad": rr,
    }


def _build_module():
    nc = bacc.Bacc("TRN2", target_bir_lowering=False, debug=False,
                   num_devices=N_CORES)

    lg = nc.dram_tensor("lg", [SPC, H, W], F32, kind="ExternalInput").ap()
    tg = nc.dram_tensor("tg", [SPC, H, W], F32, kind="ExternalInput").ap()
    band9 = nc.dram_tensor("band9", [128, 128], BF16, kind="ExternalInput").ap()
    halo_t = nc.dram_tensor("halo_t", [128, 128], BF16, kind="ExternalInput").ap()
    halo_b = nc.dram_tensor("halo_b", [128, 128], BF16, kind="ExternalInput").ap()
    onesd = nc.dram_tensor("ones", [128, 1], BF16, kind="ExternalInput").ap()
    edgeL = nc.dram_tensor("edgeL", [128, NB * 4], BF16, kind="ExternalInput").ap()
    edgeR = nc.dram_tensor("edgeR", [128, NB * 4], BF16, kind="ExternalInput").ap()
    cbiasd = nc.dram_tensor("cbias", [128, NB], F32, kind="ExternalInput").ap()
    rradd = nc.dram_tensor("rrad", [128, NB], F32, kind="ExternalInput").ap()

    o_bce = nc.dram_tensor("o_bce", [128, SPC], F32, kind="ExternalOutput").ap()
    o_sgz = nc.dram_tensor("o_sgz", [128, SPC], F32, kind="ExternalOutput").ap()
    o_g = nc.dram_tensor("o_g", [128, NSLOT], F32, kind="ExternalOutput").ap()
    o_isum = nc.dram_tensor("o_isum", [1, SPC * 512], F32,
                            kind="ExternalOutput").ap()

    with tile.TileContext(nc) as tc:
        with (
            tc.tile_pool(name="const", bufs=1) as cpool,
            tc.tile_pool(name="io", bufs=2) as iopool,
            tc.tile_pool(name="mega", bufs=1) as mpool,
            tc.tile_pool(name="ztp", bufs=2) as ztpool,
            tc.tile_pool(name="sgp", bufs=2) as sgpool,
            tc.tile_pool(name="blk", bufs=2) as bpool,
            tc.tile_pool(name="acc", bufs=1) as apool,
            tc.tile_pool(name="ps", bufs=2, space="PSUM") as pspool,
            tc.tile_pool(name="psi", bufs=2, space="PSUM") as psipool,
        ):
            # ---- constants into SBUF
            band9_t = cpool.tile([128, 128], BF16, tag="band9")
            halo_t_t = cpool.tile([128, 128], BF16, tag="halo_t")
            halo_b_t = cpool.tile([128, 128], BF16, tag="halo_b")
            ones_t = cpool.tile([128, 1], BF16, tag="ones")
            edgeL_t = cpool.tile([128, NB * 4], BF16, tag="edgeL")
            edgeR_t = cpool.tile([128, NB * 4], BF16, tag="edgeR")
            cbias_t = cpool.tile([128, NB], F32, tag="cbias")
            rrad_t = cpool.tile([128, NB], F32, tag="rrad")
            nc.sync.dma_start(out=band9_t[:], in_=band9)
            nc.sync.dma_start(out=halo_t_t[:], in_=halo_t)
            nc.sync.dma_start(out=halo_b_t[:], in_=halo_b)
            nc.sync.dma_start(out=ones_t[:], in_=onesd)
            nc.sync.dma_start(out=edgeL_t[:], in_=edgeL)
            nc.sync.dma_start(out=edgeR_t[:], in_=edgeR)
            nc.sync.dma_start(out=cbias_t[:], in_=cbiasd)
            nc.sync.dma_start(out=rrad_t[:], in_=rradd)

            # ---- accumulator staging
            a_bce = apool.tile([128, SPC], F32, tag="a_bce")
            a_sgz = apool.tile([128, SPC], F32, tag="a_sgz")
            a_g = apool.tile([128, NSLOT], F32, tag="a_g")
            i_stage = apool.tile([1, SPC * 512], F32, tag="i_stage")

            # ---- persistent chain buffers
            cA = mpool.tile([128, MEGAX], BF16, tag="cA")
            cB = mpool.tile([128, MEGAX], BF16, tag="cB")
            nc.vector.memset(cA[:, MEGA:MEGAX], 0.0)
            nc.vector.memset(cB[:, MEGA:MEGAX], 0.0)

            def load_sample(s):
                Lt = iopool.tile([128, MEGAX], BF16, tag="Lt")
                Tt = iopool.tile([128, MEGAX], BF16, tag="Tt")
                Tt3 = Tt[:, 0:MEGA].rearrange("p (b c) -> p b c", c=SEG)
                Lt3 = Lt[:, 0:MEGA].rearrange("p (b c) -> p b c", c=SEG)
                # pads: targets 0 (morphology), logits 1.0 (z pads -> +20)
                nc.vector.memset(Tt3[:, :, 0:PAD], 0.0)
                nc.vector.memset(Tt3[:, :, PAD + W:SEG], 0.0)
                nc.vector.memset(Tt[:, MEGA:MEGAX], 0.0)
                nc.vector.memset(Lt3[:, :, 0:PAD], 1.0)
                nc.vector.memset(Lt3[:, :, PAD + W:SEG], 1.0)
                nc.gpsimd.dma_start(
                    out=Lt3[:, :, PAD:PAD + W],
                    in_=lg[s].rearrange("(b p) w -> p b w", p=128))
                nc.gpsimd.dma_start(
                    out=Tt3[:, :, PAD:PAD + W],
                    in_=tg[s].rearrange("(b p) w -> p b w", p=128))
                return Lt, Tt

            def z_and_sig(s, Lt, Tt):
                # zt = (2t-1)*x megatile-wide; pads forced to +ZPAD
                zt = ztpool.tile([128, MEGA], BF16, tag="zt")
                zt3 = zt[:].rearrange("p (b c) -> p b c", c=SEG)
                nc.vector.tensor_scalar(
                    out=zt[:], in0=Tt[:, 0:MEGA],
                    scalar1=2.0, scalar2=-1.0, op0=OP.mult, op1=OP.add)
                nc.vector.memset(zt3[:, :, 0:PAD], ZPAD)
                nc.vector.memset(zt3[:, :, PAD + W:SEG], ZPAD)
                nc.vector.tensor_mul(zt[:], zt[:], Lt[:, 0:MEGA])
                sgz = sgpool.tile([128, MEGA], BF16, tag="sgz")
                nc.scalar.activation(sgz[:], zt[:], AF.Sigmoid,
                                     accum_out=a_sgz[:, s:s + 1])
                return sgz

            def ln_pass(s, sgz):
                mln = ztpool.tile([128, MEGA], BF16, tag="zt")  # reuse zt bufs
                nc.scalar.activation(mln[:], sgz[:], AF.Ln,
                                     accum_out=a_bce[:, s:s + 1])
                return mln

            def blocks(s, Lt, Tt, sgz, mln):
                # horizontal 9-box-sum (log chain), width MEGA
                CW = MEGA
                nc.vector.tensor_add(cA[:, 0:CW], Tt[:, 0:CW], Tt[:, 1:CW + 1])
                nc.vector.tensor_add(cB[:, 0:CW], cA[:, 0:CW], cA[:, 2:CW + 2])
                nc.vector.tensor_add(cA[:, 0:CW], cB[:, 0:CW], cB[:, 4:CW + 4])
                nc.vector.tensor_add(cB[:, 0:CW], cA[:, 0:CW], Tt[:, 8:CW + 8])
                s9v = cB[:, 0:MEGA].rearrange("p (b c) -> p b c", c=SEG)
                eL3 = edgeL_t[:].rearrange("p (b c) -> p b c", c=4)
                eR3 = edgeR_t[:].rearrange("p (b c) -> p b c", c=4)
                nc.vector.tensor_mul(s9v[:, :, 4:8], s9v[:, :, 4:8], eL3)
                nc.vector.tensor_mul(s9v[:, :, 1024:1028],
                                     s9v[:, :, 1024:1028], eR3)

                ipsum = psipool.tile([1, 512], F32, tag="ipsum")
                for b in range(NB):
                    o = b * SEG + PAD
                    slot = s * NB + b

                    # vertical 9-box-sum via banded matmuls into PSUM
                    S2D = pspool.tile([128, W], F32, tag="S2D")
                    mm = [(band9_t, b)]
                    if b > 0:
                        mm.append((halo_t_t, b - 1))
                    if b < NB - 1:
                        mm.append((halo_b_t, b + 1))
                    for i, (wt, bb) in enumerate(mm):
                        for h_ in range(2):
                            rc = bb * SEG + 4 + h_ * 512
                            nc.tensor.matmul(
                                S2D[:, h_ * 512:(h_ + 1) * 512],
                                wt[:], cB[:, rc:rc + 512],
                                start=(i == 0), stop=(i == len(mm) - 1))

                    # u = |S2D - c| on Scalar (PSUM -> SBUF bf16)
                    u = bpool.tile([128, W], BF16, tag="u")
                    nc.scalar.activation(u[:], S2D[:], AF.Abs,
                                         bias=cbias_t[:, b:b + 1], scale=1.0)

                    # masked bce sum: sum(mln * [u < r])
                    junk = bpool.tile([128, W], BF16, tag="junk")
                    nc.vector.scalar_tensor_tensor(
                        out=junk[:], in0=u[:], scalar=rrad_t[:, b:b + 1],
                        in1=mln[:, o:o + W], op0=OP.is_lt, op1=OP.mult,
                        accum_out=a_g[:, slot:slot + 1])

                    # intersection product t*sgz -> ones-matmul column sums
                    q = bpool.tile([128, W], BF16, tag="q")
                    nc.vector.tensor_mul(q[:], Tt[:, o:o + W],
                                         sgz[:, o:o + W])
                    nc.tensor.matmul(ipsum[:], ones_t[:], q[:, 0:512],
                                     start=(b == 0), stop=False)
                    nc.tensor.matmul(ipsum[:], ones_t[:], q[:, 512:1024],
                                     start=False, stop=(b == NB - 1))

                # stage I column sums (Scalar ACT Copy: no table switch)
                nc.scalar.activation(
                    i_stage[0:1, s * 512:(s + 1) * 512], ipsum[0:1, :],
                    AF.Copy)

            # ---- paired-sample schedule
            for p in range(SPC // 2):
                s0, s1 = 2 * p, 2 * p + 1
                Lt0, Tt0 = load_sample(s0)
                Lt1, Tt1 = load_sample(s1)
                sgz0 = z_and_sig(s0, Lt0, Tt0)
                sgz1 = z_and_sig(s1, Lt1, Tt1)
                mln0 = ln_pass(s0, sgz0)
                mln1 = ln_pass(s1, sgz1)
                blocks(s0, Lt0, Tt0, sgz0, mln0)
                blocks(s1, Lt1, Tt1, sgz1, mln1)

            nc.sync.dma_start(out=o_bce, in_=a_bce[:])
            nc.sync.dma_start(out=o_sgz, in_=a_sgz[:])
            nc.sync.dma_start(out=o_g, in_=a_g[:])
            nc.sync.dma_start(out=o_isum, in_=i_stage[:])
    nc.finalize()
    return nc


_NC = None


def _get_module():
    global _NC
    if _NC is None:
        _NC = _build_module()
    return _NC


def _run(logits, targets, trace=False):
    lg = np.ascontiguousarray(np.asarray(logits, np.float32).reshape(B, H, W))
    tg = np.ascontiguousarray(np.asarray(targets, np.float32).reshape(B, H, W))
    consts = _make_consts()
    nc = _get_module()
    in_maps = []
    for c in range(N_CORES):
        m = dict(consts)
        m["lg"] = lg[c * SPC:(c + 1) * SPC]
        m["tg"] = tg[c * SPC:(c + 1) * SPC]
        in_maps.append(m)
    res = run_bass_kernel_spmd(nc, in_maps, core_ids=list(range(N_CORES)),
                               trace=trace)
    return res


def _combine(results):
    HW = H * W
    wb = 0.0
    scores = []
    for c in range(N_CORES):
        r = results[c]
        # o_bce holds sum of ln(sigmoid(z)) = -bce (pads add ~ln(1)=0)
        # o_g holds sum of ln(sigmoid(z)) * boundary_mask
        bce_s = -r["o_bce"].astype(np.float64).sum()
        gm = -r["o_g"].astype(np.float64).sum()
        wb += bce_s + (BOUNDARY_WEIGHT - 1.0) * gm
        for s in range(SPC):
            I = r["o_isum"][0, s * 512:(s + 1) * 512].astype(np.float64).sum()
            # sigmoid accum includes 128 pad cols of exactly 1.0 per row
            sgz_sum = r["o_sgz"][:, s].astype(np.float64).sum() - 128.0 * 128.0
            # P + T = 2I - sum(sgz) + HW  (target count cancels)
            scores.append(2.0 * (I + SMOOTH) / (2.0 * I - sgz_sum + HW + SMOOTH))
    bce = wb / (B * HW)
    dice = 1.0 - np.mean(scores)
    return np.float32(bce + dice)


def kernel(logits, targets):
    res = _run(logits, targets, trace=False)
    return _combine(res.results)


# revision 4
# speedup vs baseline: 1.3641x; 1.0878x over previous
"""Trainium2 Bass kernel for nn_CombinedLoss: weighted BCE (9x9 morphology
boundary weights) + soft dice, data-parallel over 8 NeuronCores.

Self-contained: hardcodes shapes [32,1,1024,1024] f32 and the sharding
(4 samples per core). Host combines tiny per-core partial sums.

V4 design notes (per core):
 - samples processed in pairs; Scalar ACT table switches (sigmoid <->
   ln sets) batched to 2 per pair.
 - per-pair schedule overlaps engines: z(s0), z(s1) on DVE; sigmoid/ln
   ACTs on Scalar run concurrently with the horizontal box-sum chains
   (cB double-buffered so chain(s1) can run while TensorE consumes
   chain(s0)).
 - z = x*(2t-1) megatile-wide; pads forced to +20 so they contribute
   exactly 1.0 to the sigmoid accum (subtracted host-side) and ~0 to
   the ln accum.
 - boundary mask combined into ONE test: boundary <=> 0.5 < S2D < tau
   <=> |S2D - c| < r. u = |S2D - c| on Scalar (ACT Abs, per-partition
   bias), then one stt(is_lt,mult) accumulates sum(bce * mask).
 - intersection I = sum(t*sgz): after the ln pass, sgz is multiplied
   in place by t (one megatile tt), then ones-matmul column sums
   accumulate in PSUM across each sample (TensorE). This also releases
   the targets tile early so the next pair's loads prefetch.
 - dice denominator: P + T = 2I - sum(sgz) + HW (target count
   cancels): no target-sum reduction needed.
 - mln reuses the zt pool buffers (zt dead once sigmoid ran).
"""

import numpy as np
import ml_dtypes

import concourse.bass as bass
import concourse.tile as tile
from concourse import bacc, mybir
from concourse.bass_utils import run_bass_kernel_spmd

AF = mybir.ActivationFunctionType
OP = mybir.AluOpType
BF16 = mybir.dt.bfloat16
F32 = mybir.dt.float32

B, H, W = 32, 1024, 1024
N_CORES = 8
SPC = B // N_CORES          # samples per core = 4
NB = H // 128               # 8 row blocks per sample
PAD = 8
SEG = W + 2 * PAD           # 1040 padded segment stride
MEGA = NB * SEG             # 8320
MEGAX = MEGA + 8            # +8 tail so shifted reads stay in-bounds
NSLOT = SPC * NB            # 32 accum slots per core
BOUNDARY_WEIGHT = 3.0
SMOOTH = 1.0
ZPAD = 20.0                 # zt pad value: sigmoid(20) == 1.0 in bf16

NPBF16 = ml_dtypes.bfloat16


def _cnt_v(r):
    return min(r, 4) + 1 + min(H - 1 - r, 4)


def _make_consts():
    k = np.arange(128)
    band9 = (np.abs(k[:, None] - k[None, :]) <= 4).astype(NPBF16)
    ht = np.zeros((128, 128), NPBF16)   # rows from PREV block (top halo)
    hb = np.zeros((128, 128), NPBF16)   # rows from NEXT block (bottom halo)
    for m in range(4):
        ht[124 + m:, m] = 1.0
    for m in range(124, 128):
        hb[: m - 123, m] = 1.0
    ones = np.ones((128, 1), NPBF16)
    # horizontal clipped-window rescale: data cols 0..3 and 1020..1023
    el = np.tile(np.array([9 / 5, 9 / 6, 9 / 7, 9 / 8], np.float32), NB)
    er = np.tile(np.array([9 / 8, 9 / 7, 9 / 6, 9 / 5], np.float32), NB)
    edgeL = np.broadcast_to(el, (128, NB * 4)).astype(NPBF16).copy()
    edgeR = np.broadcast_to(er, (128, NB * 4)).astype(NPBF16).copy()
    # combined-threshold constants: boundary <=> |S - c| < r
    cc = np.zeros((128, NB), np.float32)
    rr = np.zeros((128, NB), np.float32)
    for b in range(NB):
        for p in range(128):
            cv = 9.0 * _cnt_v(b * 128 + p)
            cc[p, b] = -cv / 2.0            # ACT bias = -c
            rr[p, b] = (cv - 1.0) / 2.0     # compare radius
    return {
        "band9": band9, "halo_t": ht, "halo_b": hb, "ones": ones,
        "edgeL": edgeL, "edgeR": edgeR, "cbias": cc, "rrad": rr,
    }


def _build_module():
    nc = bacc.Bacc("TRN2", target_bir_lowering=False, debug=False,
                   num_devices=N_CORES)

    lg = nc.dram_tensor("lg", [SPC, H, W], F32, kind="ExternalInput").ap()
    tg = nc.dram_tensor("tg", [SPC, H, W], F32, kind="ExternalInput").ap()
    band9 = nc.dram_tensor("band9", [128, 128], BF16, kind="ExternalInput").ap()
    halo_t = nc.dram_tensor("halo_t", [128, 128], BF16, kind="ExternalInput").ap()
    halo_b = nc.dram_tensor("halo_b", [128, 128], BF16, kind="ExternalInput").ap()
    onesd = nc.dram_tensor("ones", [128, 1], BF16, kind="ExternalInput").ap()
    edgeL = nc.dram_tensor("edgeL", [128, NB * 4], BF16, kind="ExternalInput").ap()
    edgeR = nc.dram_tensor("edgeR", [128, NB * 4], BF16, kind="ExternalInput").ap()
    cbiasd = nc.dram_tensor("cbias", [128, NB], F32, kind="ExternalInput").ap()
    rradd = nc.dram_tensor("rrad", [128, NB], F32, kind="ExternalInput").ap()

    o_bce = nc.dram_tensor("o_bce", [128, SPC], F32, kind="ExternalOutput").ap()
    o_sgz = nc.dram_tensor("o_sgz", [128, SPC], F32, kind="ExternalOutput").ap()
    o_g = nc.dram_tensor("o_g", [128, NSLOT], F32, kind="ExternalOutput").ap()
    o_isum = nc.dram_tensor("o_isum", [1, SPC * 512], F32,
                            kind="ExternalOutput").ap()

    with tile.TileContext(nc) as tc:
        with (
            tc.tile_pool(name="const", bufs=1) as cpool,
            tc.tile_pool(name="io", bufs=2) as iopool,
            tc.tile_pool(name="mega", bufs=1) as mpool,
            tc.tile_pool(name="cbp", bufs=2) as cbpool,
            tc.tile_pool(name="ztp", bufs=2) as ztpool,
            tc.tile_pool(name="sgp", bufs=2) as sgpool,
            tc.tile_pool(name="blk", bufs=2) as bpool,
            tc.tile_pool(name="acc", bufs=1) as apool,
            tc.tile_pool(name="ps", bufs=2, space="PSUM") as pspool,
            tc.tile_pool(name="psi", bufs=2, space="PSUM") as psipool,
        ):
            # ---- constants into SBUF
            band9_t = cpool.tile([128, 128], BF16, tag="band9")
            halo_t_t = cpool.tile([128, 128], BF16, tag="halo_t")
            halo_b_t = cpool.tile([128, 128], BF16, tag="halo_b")
            ones_t = cpool.tile([128, 1], BF16, tag="ones")
            edgeL_t = cpool.tile([128, NB * 4], BF16, tag="edgeL")
            edgeR_t = cpool.tile([128, NB * 4], BF16, tag="edgeR")
            cbias_t = cpool.tile([128, NB], F32, tag="cbias")
            rrad_t = cpool.tile([128, NB], F32, tag="rrad")
            nc.sync.dma_start(out=band9_t[:], in_=band9)
            nc.sync.dma_start(out=halo_t_t[:], in_=halo_t)
            nc.sync.dma_start(out=halo_b_t[:], in_=halo_b)
            nc.sync.dma_start(out=ones_t[:], in_=onesd)
            nc.sync.dma_start(out=edgeL_t[:], in_=edgeL)
            nc.sync.dma_start(out=edgeR_t[:], in_=edgeR)
            nc.sync.dma_start(out=cbias_t[:], in_=cbiasd)
            nc.sync.dma_start(out=rrad_t[:], in_=rradd)

            # ---- accumulator staging
            a_bce = apool.tile([128, SPC], F32, tag="a_bce")
            a_sgz = apool.tile([128, SPC], F32, tag="a_sgz")
            a_g = apool.tile([128, NSLOT], F32, tag="a_g")
            i_stage = apool.tile([1, SPC * 512], F32, tag="i_stage")

            # ---- chain scratch (ping) — pong cB is double-buffered
            cA = mpool.tile([128, MEGAX], BF16, tag="cA")
            nc.vector.memset(cA[:, MEGA:MEGAX], 0.0)

            def load_sample(s):
                Lt = iopool.tile([128, MEGAX], BF16, tag="Lt")
                Tt = iopool.tile([128, MEGAX], BF16, tag="Tt")
                Tt3 = Tt[:, 0:MEGA].rearrange("p (b c) -> p b c", c=SEG)
                Lt3 = Lt[:, 0:MEGA].rearrange("p (b c) -> p b c", c=SEG)
                # pads: targets 0 (morphology), logits 1.0 (z pads -> +20)
                nc.vector.memset(Tt3[:, :, 0:PAD], 0.0)
                nc.vector.memset(Tt3[:, :, PAD + W:SEG], 0.0)
                nc.vector.memset(Tt[:, MEGA:MEGAX], 0.0)
                nc.vector.memset(Lt3[:, :, 0:PAD], 1.0)
                nc.vector.memset(Lt3[:, :, PAD + W:SEG], 1.0)
                nc.gpsimd.dma_start(
                    out=Lt3[:, :, PAD:PAD + W],
                    in_=lg[s].rearrange("(b p) w -> p b w", p=128))
                nc.gpsimd.dma_start(
                    out=Tt3[:, :, PAD:PAD + W],
                    in_=tg[s].rearrange("(b p) w -> p b w", p=128))
                return Lt, Tt

            def z_and_sig(s, Lt, Tt):
                # zt = (2t-1)*x megatile-wide; pads forced to +ZPAD
                zt = ztpool.tile([128, MEGA], BF16, tag="zt")
                zt3 = zt[:].rearrange("p (b c) -> p b c", c=SEG)
                nc.vector.tensor_scalar(
                    out=zt[:], in0=Tt[:, 0:MEGA],
                    scalar1=2.0, scalar2=-1.0, op0=OP.mult, op1=OP.add)
                nc.vector.memset(zt3[:, :, 0:PAD], ZPAD)
                nc.vector.memset(zt3[:, :, PAD + W:SEG], ZPAD)
                nc.vector.tensor_mul(zt[:], zt[:], Lt[:, 0:MEGA])
                sgz = sgpool.tile([128, MEGA], BF16, tag="sgz")
                nc.scalar.activation(sgz[:], zt[:], AF.Sigmoid,
                                     accum_out=a_sgz[:, s:s + 1])
                return sgz

            def ln_pass(s, sgz):
                mln = ztpool.tile([128, MEGA], BF16, tag="zt")  # reuse zt bufs
                nc.scalar.activation(mln[:], sgz[:], AF.Ln,
                                     accum_out=a_bce[:, s:s + 1])
                return mln

            def chain(Tt):
                # horizontal 9-box-sum (log chain), width MEGA
                cB = cbpool.tile([128, MEGAX], BF16, tag="cB")
                nc.vector.memset(cB[:, MEGA:MEGAX], 0.0)
                CW = MEGA
                nc.vector.tensor_add(cA[:, 0:CW], Tt[:, 0:CW], Tt[:, 1:CW + 1])
                nc.vector.tensor_add(cB[:, 0:CW], cA[:, 0:CW], cA[:, 2:CW + 2])
                nc.vector.tensor_add(cA[:, 0:CW], cB[:, 0:CW], cB[:, 4:CW + 4])
                nc.vector.tensor_add(cB[:, 0:CW], cA[:, 0:CW], Tt[:, 8:CW + 8])
                s9v = cB[:, 0:MEGA].rearrange("p (b c) -> p b c", c=SEG)
                eL3 = edgeL_t[:].rearrange("p (b c) -> p b c", c=4)
                eR3 = edgeR_t[:].rearrange("p (b c) -> p b c", c=4)
                nc.vector.tensor_mul(s9v[:, :, 4:8], s9v[:, :, 4:8], eL3)
                nc.vector.tensor_mul(s9v[:, :, 1024:1028],
                                     s9v[:, :, 1024:1028], eR3)
                return cB

            def q_inplace(Tt, sgz):
                # sgz *= t (megatile): sgz becomes the intersection product;
                # pads become 0 (t pads are 0). Releases Tt for prefetch.
                nc.vector.tensor_mul(sgz[:], Tt[:, 0:MEGA], sgz[:])

            def blocks(s, cB, sgz_q, mln):
                ipsum = psipool.tile([1, 512], F32, tag="ipsum")
                for b in range(NB):
                    o = b * SEG + PAD
                    slot = s * NB + b

                    # vertical 9-box-sum via banded matmuls into PSUM
                    S2D = pspool.tile([128, W], F32, tag="S2D")
                    mm = [(band9_t, b)]
                    if b > 0:
                        mm.append((halo_t_t, b - 1))
                    if b < NB - 1:
                        mm.append((halo_b_t, b + 1))
                    for i, (wt, bb) in enumerate(mm):
                        for h_ in range(2):
                            rc = bb * SEG + 4 + h_ * 512
                            nc.tensor.matmul(
                                S2D[:, h_ * 512:(h_ + 1) * 512],
                                wt[:], cB[:, rc:rc + 512],
                                start=(i == 0), stop=(i == len(mm) - 1))

                    # u = |S2D - c| on Scalar (PSUM -> SBUF bf16)
                    u = bpool.tile([128, W], BF16, tag="u")
                    nc.scalar.activation(u[:], S2D[:], AF.Abs,
                                         bias=cbias_t[:, b:b + 1], scale=1.0)

                    # masked bce sum: sum(mln * [u < r])
                    junk = bpool.tile([128, W], BF16, tag="junk")
                    nc.vector.scalar_tensor_tensor(
                        out=junk[:], in0=u[:], scalar=rrad_t[:, b:b + 1],
                        in1=mln[:, o:o + W], op0=OP.is_lt, op1=OP.mult,
                        accum_out=a_g[:, slot:slot + 1])

                    # intersection: ones-matmul column sums of t*sgz
                    nc.tensor.matmul(ipsum[:], ones_t[:],
                                     sgz_q[:, o:o + 512],
                                     start=(b == 0), stop=False)
                    nc.tensor.matmul(ipsum[:], ones_t[:],
                                     sgz_q[:, o + 512:o + 1024],
                                     start=False, stop=(b == NB - 1))

                # stage I column sums (ACT Copy is in every table set)
                nc.scalar.activation(
                    i_stage[0:1, s * 512:(s + 1) * 512], ipsum[0:1, :],
                    AF.Copy)

            # ---- paired-sample schedule
            for p in range(SPC // 2):
                s0, s1 = 2 * p, 2 * p + 1
                Lt0, Tt0 = load_sample(s0)
                Lt1, Tt1 = load_sample(s1)
                sgz0 = z_and_sig(s0, Lt0, Tt0)
                sgz1 = z_and_sig(s1, Lt1, Tt1)
                mln0 = ln_pass(s0, sgz0)
                mln1 = ln_pass(s1, sgz1)
                cB0 = chain(Tt0)
                q_inplace(Tt0, sgz0)
                cB1 = chain(Tt1)
                q_inplace(Tt1, sgz1)
                blocks(s0, cB0, sgz0, mln0)
                blocks(s1, cB1, sgz1, mln1)

            nc.sync.dma_start(out=o_bce, in_=a_bce[:])
            nc.sync.dma_start(out=o_sgz, in_=a_sgz[:])
            nc.sync.dma_start(out=o_g, in_=a_g[:])
            nc.sync.dma_start(out=o_isum, in_=i_stage[:])
    nc.finalize()
    return nc


_NC = None


def _get_module():
    global _NC
    if _NC is None:
        _NC = _build_module()
    return _NC


def _run(logits, targets, trace=False):
    lg = np.ascontiguousarray(np.asarray(logits, np.float32).reshape(B, H, W))
    tg = np.ascontiguousarray(np.asarray(targets, np.float32).reshape(B, H, W))
    consts = _make_consts()
    nc = _get_module()
    in_maps = []
    for c in range(N_CORES):
        m = dict(consts)
        m["lg"] = lg[c * SPC:(c + 1) * SPC]
        m["tg"] = tg[c * SPC:(c + 1) * SPC]
        in_maps.append(m)
    res = run_bass_kernel_spmd(nc, in_maps, core_ids=list(range(N_CORES)),
                               trace=trace)
    return res


def _combine(results):
    HW = H * W
    wb = 0.0
    scores = []
    for c in range(N_CORES):
        r = results[c]
        # o_bce holds sum of ln(sigmoid(z)) = -bce (pads add ~ln(1)=0)
        # o_g holds sum of ln(sigmoid(z)) * boundary_mask
        bce_s = -r["o_bce"].astype(np.float64).sum()
        gm = -r["o_g"].astype(np.float64).sum()
        wb += bce_s + (BOUNDARY_WEIGHT - 1.0) * gm
        for s in range(SPC):
            I = r["o_isum"][0, s * 512:(s + 1) * 512].astype(np.float64).sum()
            # sigmoid accum includes 128 pad cols of exactly 1.0 per row
            sgz_sum = r["o_sgz"][:, s].astype(np.float64).sum() - 128.0 * 128.0
            # P + T = 2I - sum(sgz) + HW  (target count cancels)
            scores.append(2.0 * (I + SMOOTH) / (2.0 * I - sgz_sum + HW + SMOOTH))
    bce = wb / (B * HW)
    dice = 1.0 - np.mean(scores)
    return np.float32(bce + dice)


def kernel(logits, targets):
    res = _run(logits, targets, trace=False)
    return _combine(res.results)


# revision 5
# speedup vs baseline: 1.4029x; 1.0285x over previous
"""Trainium2 Bass kernel for nn_CombinedLoss: weighted BCE (9x9 morphology
boundary weights) + soft dice, data-parallel over 8 NeuronCores.

Self-contained: hardcodes shapes [32,1,1024,1024] f32 and the sharding
(4 samples per core). Host combines tiny per-core partial sums.

V4 design notes (per core):
 - samples processed in pairs; Scalar ACT table switches (sigmoid <->
   ln sets) batched to 2 per pair.
 - per-pair schedule overlaps engines: z(s0), z(s1) on DVE; sigmoid/ln
   ACTs on Scalar run concurrently with the horizontal box-sum chains
   (cB double-buffered so chain(s1) can run while TensorE consumes
   chain(s0)).
 - z = x*(2t-1) megatile-wide; pads forced to +20 so they contribute
   exactly 1.0 to the sigmoid accum (subtracted host-side) and ~0 to
   the ln accum.
 - boundary mask combined into ONE test: boundary <=> 0.5 < S2D < tau
   <=> |S2D - c| < r. u = |S2D - c| on Scalar (ACT Abs, per-partition
   bias), then one stt(is_lt,mult) accumulates sum(bce * mask).
 - intersection I = sum(t*sgz): after the ln pass, sgz is multiplied
   in place by t (one megatile tt), then ones-matmul column sums
   accumulate in PSUM across each sample (TensorE). This also releases
   the targets tile early so the next pair's loads prefetch.
 - dice denominator: P + T = 2I - sum(sgz) + HW (target count
   cancels): no target-sum reduction needed.
 - mln reuses the zt pool buffers (zt dead once sigmoid ran).
"""

import numpy as np
import ml_dtypes

import concourse.bass as bass
import concourse.tile as tile
from concourse import bacc, mybir
from concourse.bass_utils import run_bass_kernel_spmd

AF = mybir.ActivationFunctionType
OP = mybir.AluOpType
BF16 = mybir.dt.bfloat16
F32 = mybir.dt.float32

B, H, W = 32, 1024, 1024
N_CORES = 8
SPC = B // N_CORES          # samples per core = 4
NB = H // 128               # 8 row blocks per sample
PAD = 8
SEG = W + 2 * PAD           # 1040 padded segment stride
MEGA = NB * SEG             # 8320
MEGAX = MEGA + 8            # +8 tail so shifted reads stay in-bounds
NSLOT = SPC * NB            # 32 accum slots per core
BOUNDARY_WEIGHT = 3.0
SMOOTH = 1.0
ZPAD = 20.0                 # zt pad value: sigmoid(20) == 1.0 in bf16

NPBF16 = ml_dtypes.bfloat16


def _cnt_v(r):
    return min(r, 4) + 1 + min(H - 1 - r, 4)


def _make_consts():
    k = np.arange(128)
    band9 = (np.abs(k[:, None] - k[None, :]) <= 4).astype(NPBF16)
    ht = np.zeros((128, 128), NPBF16)   # rows from PREV block (top halo)
    hb = np.zeros((128, 128), NPBF16)   # rows from NEXT block (bottom halo)
    for m in range(4):
        ht[124 + m:, m] = 1.0
    for m in range(124, 128):
        hb[: m - 123, m] = 1.0
    ones = np.ones((128, 1), NPBF16)
    # horizontal clipped-window rescale: data cols 0..3 and 1020..1023
    el = np.tile(np.array([9 / 5, 9 / 6, 9 / 7, 9 / 8], np.float32), NB)
    er = np.tile(np.array([9 / 8, 9 / 7, 9 / 6, 9 / 5], np.float32), NB)
    edgeL = np.broadcast_to(el, (128, NB * 4)).astype(NPBF16).copy()
    edgeR = np.broadcast_to(er, (128, NB * 4)).astype(NPBF16).copy()
    # combined-threshold constants: boundary <=> |S - c| < r
    cc = np.zeros((128, NB), np.float32)
    rr = np.zeros((128, NB), np.float32)
    for b in range(NB):
        for p in range(128):
            cv = 9.0 * _cnt_v(b * 128 + p)
            cc[p, b] = -cv / 2.0            # ACT bias = -c
            rr[p, b] = (cv - 1.0) / 2.0     # compare radius
    return {
        "band9": band9, "halo_t": ht, "halo_b": hb, "ones": ones,
        "edgeL": edgeL, "edgeR": edgeR, "cbias": cc, "rrad": rr,
    }


def _build_module():
    nc = bacc.Bacc("TRN2", target_bir_lowering=False, debug=False,
                   num_devices=N_CORES)

    lg = nc.dram_tensor("lg", [SPC, H, W], F32, kind="ExternalInput").ap()
    tg = nc.dram_tensor("tg", [SPC, H, W], F32, kind="ExternalInput").ap()
    band9 = nc.dram_tensor("band9", [128, 128], BF16, kind="ExternalInput").ap()
    halo_t = nc.dram_tensor("halo_t", [128, 128], BF16, kind="ExternalInput").ap()
    halo_b = nc.dram_tensor("halo_b", [128, 128], BF16, kind="ExternalInput").ap()
    onesd = nc.dram_tensor("ones", [128, 1], BF16, kind="ExternalInput").ap()
    edgeL = nc.dram_tensor("edgeL", [128, NB * 4], BF16, kind="ExternalInput").ap()
    edgeR = nc.dram_tensor("edgeR", [128, NB * 4], BF16, kind="ExternalInput").ap()
    cbiasd = nc.dram_tensor("cbias", [128, NB], F32, kind="ExternalInput").ap()
    rradd = nc.dram_tensor("rrad", [128, NB], F32, kind="ExternalInput").ap()

    o_bce = nc.dram_tensor("o_bce", [128, SPC], F32, kind="ExternalOutput").ap()
    o_sgz = nc.dram_tensor("o_sgz", [128, SPC], F32, kind="ExternalOutput").ap()
    o_g = nc.dram_tensor("o_g", [128, NSLOT], F32, kind="ExternalOutput").ap()
    o_isum = nc.dram_tensor("o_isum", [1, SPC * 512], F32,
                            kind="ExternalOutput").ap()

    with tile.TileContext(nc) as tc:
        with (
            tc.tile_pool(name="const", bufs=1) as cpool,
            tc.tile_pool(name="io", bufs=2) as iopool,
            tc.tile_pool(name="mega", bufs=1) as mpool,
            tc.tile_pool(name="cbp", bufs=2) as cbpool,
            tc.tile_pool(name="ztp", bufs=2) as ztpool,
            tc.tile_pool(name="sgp", bufs=2) as sgpool,
            tc.tile_pool(name="blk", bufs=2) as bpool,
            tc.tile_pool(name="acc", bufs=1) as apool,
            tc.tile_pool(name="ps", bufs=2, space="PSUM") as pspool,
            tc.tile_pool(name="psi", bufs=2, space="PSUM") as psipool,
        ):
            # ---- constants into SBUF
            band9_t = cpool.tile([128, 128], BF16, tag="band9")
            halo_t_t = cpool.tile([128, 128], BF16, tag="halo_t")
            halo_b_t = cpool.tile([128, 128], BF16, tag="halo_b")
            ones_t = cpool.tile([128, 1], BF16, tag="ones")
            edgeL_t = cpool.tile([128, NB * 4], BF16, tag="edgeL")
            edgeR_t = cpool.tile([128, NB * 4], BF16, tag="edgeR")
            cbias_t = cpool.tile([128, NB], F32, tag="cbias")
            rrad_t = cpool.tile([128, NB], F32, tag="rrad")
            nc.sync.dma_start(out=band9_t[:], in_=band9)
            nc.sync.dma_start(out=halo_t_t[:], in_=halo_t)
            nc.sync.dma_start(out=halo_b_t[:], in_=halo_b)
            nc.sync.dma_start(out=ones_t[:], in_=onesd)
            nc.sync.dma_start(out=edgeL_t[:], in_=edgeL)
            nc.sync.dma_start(out=edgeR_t[:], in_=edgeR)
            nc.sync.dma_start(out=cbias_t[:], in_=cbiasd)
            nc.sync.dma_start(out=rrad_t[:], in_=rradd)

            # ---- accumulator staging
            a_bce = apool.tile([128, SPC], F32, tag="a_bce")
            a_sgz = apool.tile([128, SPC], F32, tag="a_sgz")
            a_g = apool.tile([128, NSLOT], F32, tag="a_g")
            i_stage = apool.tile([1, SPC * 512], F32, tag="i_stage")

            # ---- chain scratch (ping) — pong cB is double-buffered
            cA = mpool.tile([128, MEGAX], BF16, tag="cA")
            nc.vector.memset(cA[:, MEGA:MEGAX], 0.0)

            # Warm up both io buffers: pad columns are memset ONCE per
            # physical buffer (loads only write data columns, so pads
            # persist across buffer reuse and the per-pair DMA is not
            # serialized behind V-queue memsets).
            for _ in range(2):
                Lt = iopool.tile([128, MEGAX], BF16, tag="Lt")
                Tt = iopool.tile([128, MEGAX], BF16, tag="Tt")
                Tt3 = Tt[:, 0:MEGA].rearrange("p (b c) -> p b c", c=SEG)
                Lt3 = Lt[:, 0:MEGA].rearrange("p (b c) -> p b c", c=SEG)
                nc.vector.memset(Tt3[:, :, 0:PAD], 0.0)
                nc.vector.memset(Tt3[:, :, PAD + W:SEG], 0.0)
                nc.vector.memset(Tt[:, MEGA:MEGAX], 0.0)
                nc.vector.memset(Lt3[:, :, 0:PAD], 1.0)
                nc.vector.memset(Lt3[:, :, PAD + W:SEG], 1.0)

            def load_sample(s):
                Lt = iopool.tile([128, MEGAX], BF16, tag="Lt")
                Tt = iopool.tile([128, MEGAX], BF16, tag="Tt")
                Tt3 = Tt[:, 0:MEGA].rearrange("p (b c) -> p b c", c=SEG)
                Lt3 = Lt[:, 0:MEGA].rearrange("p (b c) -> p b c", c=SEG)
                nc.gpsimd.dma_start(
                    out=Lt3[:, :, PAD:PAD + W],
                    in_=lg[s].rearrange("(b p) w -> p b w", p=128))
                nc.gpsimd.dma_start(
                    out=Tt3[:, :, PAD:PAD + W],
                    in_=tg[s].rearrange("(b p) w -> p b w", p=128))
                return Lt, Tt

            def z_and_sig(s, Lt, Tt):
                # zt = (2t-1)*x megatile-wide; pads forced to +ZPAD
                zt = ztpool.tile([128, MEGA], BF16, tag="zt")
                zt3 = zt[:].rearrange("p (b c) -> p b c", c=SEG)
                nc.vector.tensor_scalar(
                    out=zt[:], in0=Tt[:, 0:MEGA],
                    scalar1=2.0, scalar2=-1.0, op0=OP.mult, op1=OP.add)
                nc.vector.memset(zt3[:, :, 0:PAD], ZPAD)
                nc.vector.memset(zt3[:, :, PAD + W:SEG], ZPAD)
                nc.vector.tensor_mul(zt[:], zt[:], Lt[:, 0:MEGA])
                sgz = sgpool.tile([128, MEGA], BF16, tag="sgz")
                nc.scalar.activation(sgz[:], zt[:], AF.Sigmoid,
                                     accum_out=a_sgz[:, s:s + 1])
                return sgz

            def ln_pass(s, sgz):
                mln = ztpool.tile([128, MEGA], BF16, tag="zt")  # reuse zt bufs
                nc.scalar.activation(mln[:], sgz[:], AF.Ln,
                                     accum_out=a_bce[:, s:s + 1])
                return mln

            def chain(Tt):
                # horizontal 9-box-sum (log chain), width MEGA
                cB = cbpool.tile([128, MEGAX], BF16, tag="cB")
                nc.vector.memset(cB[:, MEGA:MEGAX], 0.0)
                CW = MEGA
                nc.vector.tensor_add(cA[:, 0:CW], Tt[:, 0:CW], Tt[:, 1:CW + 1])
                nc.vector.tensor_add(cB[:, 0:CW], cA[:, 0:CW], cA[:, 2:CW + 2])
                nc.vector.tensor_add(cA[:, 0:CW], cB[:, 0:CW], cB[:, 4:CW + 4])
                nc.vector.tensor_add(cB[:, 0:CW], cA[:, 0:CW], Tt[:, 8:CW + 8])
                s9v = cB[:, 0:MEGA].rearrange("p (b c) -> p b c", c=SEG)
                eL3 = edgeL_t[:].rearrange("p (b c) -> p b c", c=4)
                eR3 = edgeR_t[:].rearrange("p (b c) -> p b c", c=4)
                nc.vector.tensor_mul(s9v[:, :, 4:8], s9v[:, :, 4:8], eL3)
                nc.vector.tensor_mul(s9v[:, :, 1024:1028],
                                     s9v[:, :, 1024:1028], eR3)
                return cB

            def q_inplace(Tt, sgz):
                # sgz *= t (megatile): sgz becomes the intersection product;
                # pads become 0 (t pads are 0). Releases Tt for prefetch.
                nc.vector.tensor_mul(sgz[:], Tt[:, 0:MEGA], sgz[:])

            def blocks(s, cB, sgz_q, mln):
                ipsum = psipool.tile([1, 512], F32, tag="ipsum")
                for b in range(NB):
                    o = b * SEG + PAD
                    slot = s * NB + b

                    # vertical 9-box-sum via banded matmuls into PSUM
                    S2D = pspool.tile([128, W], F32, tag="S2D")
                    mm = [(band9_t, b)]
                    if b > 0:
                        mm.append((halo_t_t, b - 1))
                    if b < NB - 1:
                        mm.append((halo_b_t, b + 1))
                    for i, (wt, bb) in enumerate(mm):
                        for h_ in range(2):
                            rc = bb * SEG + 4 + h_ * 512
                            nc.tensor.matmul(
                                S2D[:, h_ * 512:(h_ + 1) * 512],
                                wt[:], cB[:, rc:rc + 512],
                                start=(i == 0), stop=(i == len(mm) - 1))

                    # u = |S2D - c| on Scalar (PSUM -> SBUF bf16)
                    u = bpool.tile([128, W], BF16, tag="u")
                    nc.scalar.activation(u[:], S2D[:], AF.Abs,
                                         bias=cbias_t[:, b:b + 1], scale=1.0)

                    # masked bce sum: sum(mln * [u < r])
                    junk = bpool.tile([128, W], BF16, tag="junk")
                    nc.vector.scalar_tensor_tensor(
                        out=junk[:], in0=u[:], scalar=rrad_t[:, b:b + 1],
                        in1=mln[:, o:o + W], op0=OP.is_lt, op1=OP.mult,
                        accum_out=a_g[:, slot:slot + 1])

                    # intersection: ones-matmul column sums of t*sgz
                    nc.tensor.matmul(ipsum[:], ones_t[:],
                                     sgz_q[:, o:o + 512],
                                     start=(b == 0), stop=False)
                    nc.tensor.matmul(ipsum[:], ones_t[:],
                                     sgz_q[:, o + 512:o + 1024],
                                     start=False, stop=(b == NB - 1))

                # stage I column sums (ACT Copy is in every table set)
                nc.scalar.activation(
                    i_stage[0:1, s * 512:(s + 1) * 512], ipsum[0:1, :],
                    AF.Copy)

            # ---- paired-sample schedule
            for p in range(SPC // 2):
                s0, s1 = 2 * p, 2 * p + 1
                Lt0, Tt0 = load_sample(s0)
                Lt1, Tt1 = load_sample(s1)
                sgz0 = z_and_sig(s0, Lt0, Tt0)
                sgz1 = z_and_sig(s1, Lt1, Tt1)
                mln0 = ln_pass(s0, sgz0)
                mln1 = ln_pass(s1, sgz1)
                cB0 = chain(Tt0)
                q_inplace(Tt0, sgz0)
                cB1 = chain(Tt1)
                q_inplace(Tt1, sgz1)
                blocks(s0, cB0, sgz0, mln0)
                blocks(s1, cB1, sgz1, mln1)

            nc.sync.dma_start(out=o_bce, in_=a_bce[:])
            nc.sync.dma_start(out=o_sgz, in_=a_sgz[:])
            nc.sync.dma_start(out=o_g, in_=a_g[:])
            nc.sync.dma_start(out=o_isum, in_=i_stage[:])
    nc.finalize()
    return nc


_NC = None


def _get_module():
    global _NC
    if _NC is None:
        _NC = _build_module()
    return _NC


def _run(logits, targets, trace=False):
    lg = np.ascontiguousarray(np.asarray(logits, np.float32).reshape(B, H, W))
    tg = np.ascontiguousarray(np.asarray(targets, np.float32).reshape(B, H, W))
    consts = _make_consts()
    nc = _get_module()
    in_maps = []
    for c in range(N_CORES):
        m = dict(consts)
        m["lg"] = lg[c * SPC:(c + 1) * SPC]
        m["tg"] = tg[c * SPC:(c + 1) * SPC]
        in_maps.append(m)
    res = run_bass_kernel_spmd(nc, in_maps, core_ids=list(range(N_CORES)),
                               trace=trace)
    return res


def _combine(results):
    HW = H * W
    wb = 0.0
    scores = []
    for c in range(N_CORES):
        r = results[c]
        # o_bce holds sum of ln(sigmoid(z)) = -bce (pads add ~ln(1)=0)
        # o_g holds sum of ln(sigmoid(z)) * boundary_mask
        bce_s = -r["o_bce"].astype(np.float64).sum()
        gm = -r["o_g"].astype(np.float64).sum()
        wb += bce_s + (BOUNDARY_WEIGHT - 1.0) * gm
        for s in range(SPC):
            I = r["o_isum"][0, s * 512:(s + 1) * 512].astype(np.float64).sum()
            # sigmoid accum includes 128 pad cols of exactly 1.0 per row
            sgz_sum = r["o_sgz"][:, s].astype(np.float64).sum() - 128.0 * 128.0
            # P + T = 2I - sum(sgz) + HW  (target count cancels)
            scores.append(2.0 * (I + SMOOTH) / (2.0 * I - sgz_sum + HW + SMOOTH))
    bce = wb / (B * HW)
    dice = 1.0 - np.mean(scores)
    return np.float32(bce + dice)


def kernel(logits, targets):
    res = _run(logits, targets, trace=False)
    return _combine(res.results)
